# revision 1
# baseline (speedup 1.0000x reference)
"""KL-divergence loss kernel (C51 categorical projection + batchmean KL) for TRN2.

Math: the reference projects `anchor` through a C51 projection whose skew is a
compile-time scalar, so the projection collapses to a constant linear map:

    t[:, 0]  = 0
    t[:, 1]  = 0.75*a[:, 0]
    t[:, j]  = 0.75*a[:, j-1] + 0.25*a[:, j-2]          (2 <= j <= 49)
    t[:, 50] = 0.25*a[:, 48] + a[:, 49] + a[:, 50]

and the loss is sum(t * (log t - log(f + 1e-16))) / B  (terms with t==0 are 0).

Kernel strategy (pure data parallel over 8 cores, batch-sharded):
  s = 4t built with one wide fused scalar_tensor_tensor (s_j = 3*a_{j-1} + a_{j-2})
  lt = Ln(s + 1e-35)           [ScalarE, fused affine]
  lf = Ln(4f + 4e-16)          [ScalarE, fused affine]  (folds the 1/4 scale of s)
  d  = lt - lf  -> bf16        [VectorE]
  sum s*d via TensorE: accumulate lhsT=s_blk, rhs=d_blk matmuls into one
  [128,128] PSUM tile; only the diagonal is meaningful. Host sums diagonals
  of the 8 per-core results and scales by 0.25/B.
"""

import os
import numpy as np

B_TOTAL = 524288
ATOMS = 51
N_CORES = 8
ROWS_PER_CORE = B_TOTAL // N_CORES  # 65536
P = 128
R = 64  # rows per partition per tile
TILE_COLS = R * ATOMS  # 3264
N_TILES = ROWS_PER_CORE // (P * R)  # 8
MM_BLOCK = 128

_BUILT = None
_LAST_RESULTS = None


def _build():
    from contextlib import ExitStack

    import concourse.bacc as bacc
    import concourse.tile as tile
    from concourse import mybir

    nc = bacc.Bacc("TRN2", num_devices=N_CORES)

    a_dram = nc.dram_tensor(
        "anchor", [ROWS_PER_CORE, ATOMS], mybir.dt.float32, kind="ExternalInput"
    )
    f_dram = nc.dram_tensor(
        "feature", [ROWS_PER_CORE, ATOMS], mybir.dt.float32, kind="ExternalInput"
    )
    out_dram = nc.dram_tensor(
        "out", [P, MM_BLOCK], mybir.dt.float32, kind="ExternalOutput"
    )

    a_t = a_dram.ap().rearrange("(n p q) m -> n p (q m)", p=P, q=R)
    f_t = f_dram.ap().rearrange("(n p q) m -> n p (q m)", p=P, q=R)

    mult = mybir.AluOpType.mult
    add = mybir.AluOpType.add

    n_blk_full, tail = divmod(TILE_COLS, MM_BLOCK)  # 25, 64
    blocks = [(b * MM_BLOCK, MM_BLOCK) for b in range(n_blk_full)]
    if tail:
        blocks.append((n_blk_full * MM_BLOCK, tail))
    total_mms = N_TILES * len(blocks)

    with tile.TileContext(nc) as tc:
        with ExitStack() as ctx:
            a_pool = ctx.enter_context(tc.tile_pool(name="a", bufs=2))
            f_pool = ctx.enter_context(tc.tile_pool(name="f", bufs=2))
            s_pool = ctx.enter_context(tc.tile_pool(name="s", bufs=2))
            lt_pool = ctx.enter_context(tc.tile_pool(name="lt", bufs=2))
            lf_pool = ctx.enter_context(tc.tile_pool(name="lf", bufs=2))
            d_pool = ctx.enter_context(tc.tile_pool(name="d", bufs=2))
            tmp_pool = ctx.enter_context(tc.tile_pool(name="tmp", bufs=2))
            out_pool = ctx.enter_context(tc.tile_pool(name="outp", bufs=1))
            psum_pool = ctx.enter_context(
                tc.tile_pool(name="acc", bufs=1, space="PSUM")
            )

            acc = psum_pool.tile([P, MM_BLOCK], mybir.dt.float32)

            eps_s = out_pool.tile([P, 1], mybir.dt.float32, tag="eps_s")
            eps_f = out_pool.tile([P, 1], mybir.dt.float32, tag="eps_f")
            nc.gpsimd.memset(eps_s[:], 1e-35)
            nc.gpsimd.memset(eps_f[:], 4e-16)

            mm = 0
            for i in range(N_TILES):
                a_sb = a_pool.tile([P, TILE_COLS], mybir.dt.float32)
                f_sb = f_pool.tile([P, TILE_COLS], mybir.dt.float32)
                nc.sync.dma_start(out=a_sb[:], in_=a_t[i])
                nc.sync.dma_start(out=f_sb[:], in_=f_t[i])

                s_sb = s_pool.tile([P, TILE_COLS], mybir.dt.bfloat16)
                lt_sb = lt_pool.tile([P, TILE_COLS], mybir.dt.float32)
                lf_sb = lf_pool.tile([P, TILE_COLS], mybir.dt.float32)
                d_sb = d_pool.tile([P, TILE_COLS], mybir.dt.bfloat16)
                tmp = tmp_pool.tile([P, R], mybir.dt.float32)

                a3 = a_sb[:].rearrange("p (q m) -> p q m", m=ATOMS)
                s3 = s_sb[:].rearrange("p (q m) -> p q m", m=ATOMS)

                # s_j = 3*a_{j-1} + a_{j-2} for j in 2..49
                nc.vector.scalar_tensor_tensor(
                    out=s3[:, :, 2:50],
                    in0=a3[:, :, 1:49],
                    scalar=3.0,
                    in1=a3[:, :, 0:48],
                    op0=mult,
                    op1=add,
                )
                # s_1 = 3*a_0 ; s_0 = 0
                nc.vector.tensor_scalar_mul(s3[:, :, 1], a3[:, :, 0], 3.0)
                nc.gpsimd.memset(s3[:, :, 0], 0.0)
                # s_50 = a_48 + 4*a_49 + 4*a_50
                nc.vector.scalar_tensor_tensor(
                    out=tmp[:],
                    in0=a3[:, :, 49],
                    scalar=4.0,
                    in1=a3[:, :, 48],
                    op0=mult,
                    op1=add,
                )
                nc.vector.scalar_tensor_tensor(
                    out=s3[:, :, 50],
                    in0=a3[:, :, 50],
                    scalar=4.0,
                    in1=tmp[:],
                    op0=mult,
                    op1=add,
                )

                # lt = Ln(s + 1e-35); lf = Ln(4f + 4e-16)
                nc.scalar.activation(
                    out=lt_sb[:],
                    in_=s_sb[:],
                    func=mybir.ActivationFunctionType.Ln,
                    bias=eps_s[:],
                    scale=1.0,
                )
                nc.scalar.activation(
                    out=lf_sb[:],
                    in_=f_sb[:],
                    func=mybir.ActivationFunctionType.Ln,
                    bias=eps_f[:],
                    scale=4.0,
                )
                nc.vector.tensor_sub(d_sb[:], lt_sb[:], lf_sb[:])

                for c0, w in blocks:
                    nc.tensor.matmul(
                        acc[0:w, 0:w],
                        s_sb[:, c0 : c0 + w],
                        d_sb[:, c0 : c0 + w],
                        start=(mm == 0),
                        stop=(mm == total_mms - 1),
                    )
                    mm += 1

            out_sb = out_pool.tile([P, MM_BLOCK], mybir.dt.float32)
            nc.vector.tensor_copy(out_sb[:], acc[:])
            nc.sync.dma_start(out=out_dram.ap(), in_=out_sb[:])

    nc.compile()
    return nc


def kernel(anchor: np.ndarray, feature: np.ndarray) -> np.ndarray:
    global _BUILT, _LAST_RESULTS
    from concourse import bass_utils

    if _BUILT is None:
        _BUILT = _build()
    nc = _BUILT

    anchor = np.ascontiguousarray(anchor, dtype=np.float32)
    feature = np.ascontiguousarray(feature, dtype=np.float32)

    in_maps = []
    for c in range(N_CORES):
        lo, hi = c * ROWS_PER_CORE, (c + 1) * ROWS_PER_CORE
        in_maps.append({"anchor": anchor[lo:hi], "feature": feature[lo:hi]})

    res = bass_utils.run_bass_kernel_spmd(
        nc,
        in_maps,
        core_ids=list(range(N_CORES)),
        trace=bool(os.environ.get("BASS_TRACE")),
    )
    _LAST_RESULTS = res

    total = 0.0
    for c in range(N_CORES):
        total += np.trace(res.results[c]["out"].astype(np.float64))
    val = 0.25 * total / B_TOTAL
    return np.float32(val)

